# revision 10
# baseline (speedup 1.0000x reference)
"""Content-addressed cache-select kernel for Trainium2 (8 NeuronCores, SPMD).

Problem: out = cached_outputs[idx] where idx is the first row of
`fingerprints` (6x4) exactly equal to the first 4 floats of `x`, else 0.

Strategy (row-parallel over 8 cores):
  - Each core receives its 2048-row shard of all 6 cached slabs plus a
    small staged "meta" block (fingerprints, the replicated probe tiled
    x6, and index weights 0..5) packed on the host.
  - On device: one DMA stages meta into SBUF; the vector engine computes
    idx in 4 small int32 ops (bitwise equality == float equality for
    these inputs): eq = (fps == probe), hit = min-reduce per case,
    score = hit * [0..5], idx = max(score) (0 when no match, matching
    the reference's argmax-of-all-False). The SP and ACT engines
    reg-load idx and issue dynamic-offset DRAM->DRAM DMAs copying the
    selected 32MB slab shard to the output.
  - The copy is bound by the per-NC HBM path (~21 GB/s per SDMA engine
    when all 16 run, ~690 GB/s r+w aggregate): ~101.5us for
    32 MiB read + 32 MiB write. Descriptor split: part A (contiguous,
    SP queue) gives every engine 17 64KB descriptors - with a 16-desc
    starter DMA first so the SDMA doorbell lands ~0.5us early; part B
    (4-row-block interleaved AP, ACT queue) adds 15 more to all 16
    engines. The fully balanced 32-desc load finishes within ~0.8us
    across engines (profiled), beating the earlier engine-15 half-load
    hedge by ~2us.
  - Raw engine streams without nc.Block: no per-engine block branches,
    exit drains, or bass exit barrier (NRT's injected postamble barrier
    already serializes program end). The framework's const-AP memsets
    and init all-engine barrier are suppressed during Bass construction;
    they emit dead instructions ahead of the kernel body (the init
    gpsimd sem_clear is still ordered by the NRT pseudo sync barrier).
"""
import contextlib

import numpy as np

import concourse.bass as bass
import concourse.mybir as mybir
from concourse.bass_utils import run_bass_kernel_spmd

N_CASES = 6
ROWS, COLS = 16384, 4096
N_CORES = 8
RS = ROWS // N_CORES  # rows per core

# Part A (SP queue): 1088 rows = 16 groups x 68 rows -> 17 x 64KB
# descriptors per engine, issued as a 64-row starter (1 desc/engine)
# plus the remaining 1024 rows. Part B (ACT queue): 960 rows dealt as
# 4-row blocks round-robin to all 16 engines -> 15 x 64KB each, so
# every engine carries 32 descriptors. (Profiled runs never reproduced
# the prior session's intermittently-slow engine 15; the balanced
# split takes one descriptor off the stably-slower engines 0-7, which
# otherwise define the makespan.)
ROWS_A = 1088
ROWS_A0 = 64
ROWS_B = RS - ROWS_A  # 960
GROUPS_B = 16


@contextlib.contextmanager
def _lean_bass_init():
    """Suppress the framework's const-AP memsets and init all-engine
    barrier while constructing Bass. Neither is needed here: the const
    APs have no readers in this program, and the NRT pseudo sync
    barrier emitted earlier in init already orders the gpsimd semaphore
    clear against every engine's kernel body."""
    orig_barrier = bass.Bass.all_engine_barrier
    orig_memset = bass.BassGpSimd.memset
    bass.Bass.all_engine_barrier = lambda self, *a, **k: None
    bass.BassGpSimd.memset = lambda self, ap, c: None
    try:
        yield
    finally:
        bass.Bass.all_engine_barrier = orig_barrier
        bass.BassGpSimd.memset = orig_memset


def build():
    with _lean_bass_init():
        nc = bass.Bass()
    f32 = mybir.dt.float32
    i32 = mybir.dt.int32

    meta = nc.dram_tensor("meta", [2, 32], i32, kind="ExternalInput")
    cached = nc.dram_tensor("cached", [N_CASES, RS, COLS], f32, kind="ExternalInput")
    out = nc.dram_tensor("out", [RS, COLS], f32, kind="ExternalOutput")

    stage = nc.sbuf_tensor("stage", [1, 128], i32).__enter__()
    ssem = nc.semaphore("ssem").__enter__()
    vsem = nc.semaphore("vsem").__enter__()
    bsem = nc.semaphore("bsem").__enter__()
    asem = nc.semaphore("asem").__enter__()

    sync, vector, scalar = nc.sync, nc.vector, nc.scalar
    st = stage

    # Stage meta into SBUF partition 0 (DVE operands must not carry a
    # partition offset, so everything lives on one partition).
    sync.dma_start(st[0:1, 0:64], meta[0:2, 0:32]).then_inc(ssem, 16)

    vector.wait_ge(ssem, 16)
    step = [0]

    def chain(inst):
        # Same-engine RAW hazard fence: DVE is pipelined, so each op
        # waits for the previous one's semaphore before reading its
        # output.
        step[0] += 1
        inst.then_inc(vsem, 1)
        vector.wait_ge(vsem, step[0])

    # eq[64:88] = (fps == probe_tiled) as int32 0/1
    chain(
        vector.tensor_tensor(
            st[0:1, 64:88], st[0:1, 0:24], st[0:1, 24:48], mybir.AluOpType.is_equal
        )
    )
    # hit[88:94] = min over each fingerprint's 4 equality bits
    eq_v = st[0:1, 64:88].rearrange("p (a b) -> p a b", a=6)
    chain(
        vector.tensor_reduce(
            st[0:1, 88:94], eq_v, mybir.AxisListType.X, mybir.AluOpType.min
        )
    )
    # score[94:100] = hit * [0,1,2,3,4,5] (weights staged at [48:54])
    chain(
        vector.tensor_tensor(
            st[0:1, 94:100], st[0:1, 88:94], st[0:1, 48:54], mybir.AluOpType.mult
        )
    )
    # idx[100] = max(score): the matching case index, 0 if no match.
    chain(
        vector.tensor_reduce(
            st[0:1, 100:101],
            st[0:1, 94:100],
            mybir.AxisListType.X,
            mybir.AluOpType.max,
        )
    )

    def interleaved(ap, groups, f):
        # [r, COLS] region traversed as [groups, m, f*COLS]: 4-row (64KB)
        # blocks are dealt round-robin to `groups` outer slots, and the
        # strided outer dim survives AP optimization, pinning the SDMA
        # engine grouping to engines 0..groups-1. Same pattern on both
        # sides of the DMA keeps the element mapping the identity.
        if len(ap.shape) == 3:  # dynamic [1, r, COLS] slice of cached
            return ap.rearrange("q (m x f) c -> (q x) m (f c)", x=groups, f=f)
        return ap.rearrange("(m x f) c -> x m (f c)", x=groups, f=f)

    def load_idx(eng, name):
        with eng.register(name) as r:
            eng.reg_load(r, st[0:1, 100:101])
            return eng.snap(r, donate=True, min_val=0, max_val=N_CASES - 1)

    sync.wait_ge(vsem, 4)
    idx = load_idx(sync, "idxr")
    r0 = slice(0, ROWS_A0)
    sync.dma_start(out[r0, :], cached[bass.ds(idx, 1), r0, :]).then_inc(bsem, 16)
    r1 = slice(ROWS_A0, ROWS_A)
    sync.dma_start(out[r1, :], cached[bass.ds(idx, 1), r1, :]).then_inc(bsem, 16)

    scalar.wait_ge(vsem, 4)
    idx2 = load_idx(scalar, "idxa")
    rows = slice(ROWS_A, RS)
    f = 4 if COLS == 4096 else 1
    scalar.dma_start(
        interleaved(out[rows, :], GROUPS_B, f),
        interleaved(cached[bass.ds(idx2, 1), rows, :], GROUPS_B, f),
    ).then_inc(asem, 16)

    sync.wait_ge(bsem, 32)
    scalar.wait_ge(asem, 16)

    return nc


def make_meta(probe, fps):
    flat = np.zeros(64, dtype=np.int32)
    flat[0:24] = fps.reshape(-1).view(np.int32)
    flat[24:48] = np.tile(probe.reshape(-1), 6).view(np.int32)
    flat[48:54] = np.arange(6, dtype=np.int32)
    return flat.reshape(2, 32)


def run(inputs, trace=False, **spmd_kwargs):
    x = np.asarray(inputs["x"], dtype=np.float32)
    fingerprints = np.asarray(inputs["fingerprints"], dtype=np.float32)
    cached_outputs = np.asarray(inputs["cached_outputs"], dtype=np.float32)

    nc = build()
    meta = make_meta(x.reshape(-1)[:4], fingerprints)
    in_maps = []
    for c in range(N_CORES):
        shard = np.ascontiguousarray(cached_outputs[:, c * RS : (c + 1) * RS, :])
        in_maps.append({"meta": meta, "cached": shard})

    res = run_bass_kernel_spmd(
        nc, in_maps, list(range(N_CORES)), trace=trace, **spmd_kwargs
    )
    out = np.concatenate([res.results[c]["out"] for c in range(N_CORES)], axis=0)
    return out.astype(np.float32), res


def kernel(**inputs) -> np.ndarray:
    out, _ = run(inputs, trace=False)
    return out
